# revision 12
# baseline (speedup 1.0000x reference)
"""Expert-parallel MoE (top-2 of 8) kernel for 8 Trainium2 NeuronCores.

Strategy (per sharding hint): expert-parallel — expert e's FFN weights live on
core e. The (tiny) router runs on host; tokens are dispatched to their top-2
experts' cores as padded batches, each core runs its expert's gated-GLU FFN on
its batch (float32r matmuls, weights streamed from HBM), and the host applies
the routing weights and combines the per-expert partial sums.

Device layout is feature-major ([feature, token]) throughout so the contraction
dim is always on SBUF partitions and the gate_up bias is a per-partition scalar:

    XT[H=1024, C] --MM1--> GU[4096, C] --bias/clamp/silu--> ACT[2048, C]
       --MM2--> YT[1024, C]

The 1/1.702 from silu(1.702*z) = 1.702*z*sigmoid(1.702*z) is folded into
down_proj on the host. down_bias is combined on the host (it is outside the
matmuls: sum_k w_k * b2[e_k]). Weights are re-laid-out on the host so every
weight DMA moves multi-KB contiguous lines per partition.
"""

import ml_dtypes
import numpy as np

import concourse.bass as bass  # noqa: F401  (registers engines)
import concourse.mybir as mybir
import concourse.tile as tile
from concourse import bacc
from concourse.bass_utils import run_bass_kernel_spmd

ALPHA = 1.702
LIMIT = 7.0
TOP_K = 2
H = 1024
E = 8
I = 2048
F32 = mybir.dt.float32
F32R = mybir.dt.float32r
BF16 = mybir.dt.bfloat16

_prog_cache: dict = {}
last_exec_time_ns = None


def _install_ntff_hook():
    """Register the axon NTFF profiling hook if the image's antenv lacks it."""
    import sys, types  # noqa: PLC0415

    if "antenv.axon_hooks" in sys.modules:
        return
    try:
        import antenv  # noqa: PLC0415
        from trn_agent_boot.trn_boot import _ntff_profile_via_ctypes  # noqa: PLC0415

        hooks = types.ModuleType("antenv.axon_hooks")
        _h = [_ntff_profile_via_ctypes("/opt/axon/libaxon_pjrt.so")]
        hooks.set_axon_ntff_profile_hook = lambda h: _h.__setitem__(0, h)
        hooks.get_axon_ntff_profile_hook = lambda: _h[0]
        sys.modules["antenv.axon_hooks"] = hooks
        antenv.axon_hooks = hooks
    except Exception:
        pass


def _chunks(C):
    """Split C tokens into moving-operand chunks (<=512 each, >=256 when
    possible so float32r matmuls run at 1 cycle/row)."""
    n = max(1, -(-C // 512))
    base = C // n
    sizes = [base + (1 if i < C - base * n else 0) for i in range(n)]
    out, s = [], 0
    for sz in sizes:
        out.append((s, sz))
        s += sz
    return out


def _build_program(C):
    chunks = _chunks(C)
    add, mn, mx = mybir.AluOpType.add, mybir.AluOpType.min, mybir.AluOpType.max

    KH = H // 128   # 8 k-tiles over H (MM1 contraction)
    NI = I // 128   # 16 i-tiles over I (MM2 contraction)
    NJ = I // 128   # 16 gate col-tiles (up tile index = NJ + j)
    NH = H // 128   # 8 output h-tiles (MM2 stationary)

    nc = bacc.Bacc(
        "TRN2",
        target_bir_lowering=False,
        debug=False,
        enable_asserts=False,
        num_devices=E,
    )
    # host-prepared layouts (see kernel()):
    #   xt: X^T                        [H, C]
    #   w1: [m, p, k*128+c] m=0..31    (m<16: gate col-tile m; m>=16: up tile)
    #   w2: [h, p, i*128+c] h=0..7     (stationary tiles for MM2)
    #   b1: [p, m]                     per-partition bias for col-tile m
    xt_d = nc.dram_tensor("xt", [H, C], BF16, kind="ExternalInput").ap()
    w1_d = nc.dram_tensor("w1", [2 * NJ, 128, KH, 128], BF16, kind="ExternalInput").ap()
    b1_d = nc.dram_tensor("b1", [128, 2 * NJ], F32, kind="ExternalInput").ap()
    w2_d = nc.dram_tensor("w2", [NH, 128, NI, 128], BF16, kind="ExternalInput").ap()
    out_d = nc.dram_tensor("out", [H, C], F32, kind="ExternalOutput").ap()
    C2 = C // 2

    with tile.TileContext(nc) as tc:
        from contextlib import ExitStack

        with ExitStack() as ctx:
            const = ctx.enter_context(tc.tile_pool(name="const", bufs=1))
            xt_sb = const.tile([128, KH, C], BF16, tag="xt")
            for k in range(KH):
                # two DMAs per row-block: spread the fill over more DMA queues
                nc.sync.dma_start(
                    xt_sb[:, k, 0:C2], xt_d[k * 128:(k + 1) * 128, 0:C2])
                nc.sync.dma_start(
                    xt_sb[:, k, C2:C], xt_d[k * 128:(k + 1) * 128, C2:C])
            b1_sb = const.tile([128, 2 * NJ], F32, tag="b1")
            nc.sync.dma_start(b1_sb[:], b1_d[:])
            act_sb = const.tile([128, NI, C], BF16, tag="act")

            w1_pool = ctx.enter_context(tc.tile_pool(name="w1", bufs=4))
            w2_pool = ctx.enter_context(tc.tile_pool(name="w2", bufs=NH))
            ps_pool = ctx.enter_context(tc.tile_pool(name="ps", bufs=2, space="PSUM"))
            glu_pool = ctx.enter_context(tc.tile_pool(name="glu", bufs=3))
            w2_tiles = {}

            # ---- MM1 + GLU: act[:, j, :] for j in 0..15 ----
            for j in range(NJ):
                # weight loads are quartered (per-queue BW is ~25 GB/s) and
                # issued from the otherwise-idle GpSimd engine
                w1g = w1_pool.tile([128, KH, 128], BF16, tag="w1g")
                for q in range(2):
                    nc.gpsimd.dma_start(
                        w1g[:, 4 * q:4 * q + 4, :], w1_d[j, :, 4 * q:4 * q + 4, :])
                w1u = w1_pool.tile([128, KH, 128], BF16, tag="w1u")
                for q in range(2):
                    nc.sync.dma_start(
                        w1u[:, 4 * q:4 * q + 4, :], w1_d[NJ + j, :, 4 * q:4 * q + 4, :])
                if j % 2 == 1:
                    # stagger MM2 weight loads through the MM1 phase
                    h = j // 2
                    w2t = w2_pool.tile([128, NI, 128], BF16, tag="w2")
                    for q in range(4):
                        eng = nc.sync if q % 2 else nc.gpsimd
                        eng.dma_start(
                            w2t[:, 4 * q:4 * q + 4, :], w2_d[h, :, 4 * q:4 * q + 4, :])
                    w2_tiles[h] = w2t
                for (s0, sz) in chunks:
                    pg = ps_pool.tile([128, sz], F32, tag="pg")
                    for k in range(KH):
                        nc.tensor.matmul(
                            pg[:],
                            w1g[:, k, :],
                            xt_sb[:, k, s0:s0 + sz],
                            start=(k == 0),
                            stop=(k == KH - 1),
                        )
                    pu = ps_pool.tile([128, sz], F32, tag="pu")
                    for k in range(KH):
                        nc.tensor.matmul(
                            pu[:],
                            w1u[:, k, :],
                            xt_sb[:, k, s0:s0 + sz],
                            start=(k == 0),
                            stop=(k == KH - 1),
                        )
                    # gate: z = min(gu + b1g, LIMIT); glu' = silu(ALPHA*z) = ALPHA*glu
                    zg = glu_pool.tile([128, sz], F32, tag="zg")
                    nc.vector.tensor_scalar(
                        zg[:], pg[:], b1_sb[:, j:j + 1], LIMIT, op0=add, op1=mn
                    )
                    glu = glu_pool.tile([128, sz], F32, tag="glut")
                    nc.scalar.activation(
                        glu[:], zg[:], mybir.ActivationFunctionType.Silu, scale=ALPHA
                    )
                    # up: z = clip(gu + b1u, -LIMIT, LIMIT) + 1
                    zu = glu_pool.tile([128, sz], F32, tag="zu")
                    nc.vector.tensor_scalar(
                        zu[:], pu[:], b1_sb[:, NJ + j:NJ + j + 1], LIMIT,
                        op0=add, op1=mn,
                    )
                    zu2 = glu_pool.tile([128, sz], F32, tag="zu2")
                    nc.gpsimd.tensor_scalar(
                        zu2[:], zu[:], -LIMIT, 1.0, op0=mx, op1=add
                    )
                    nc.vector.tensor_mul(act_sb[:, j, s0:s0 + sz], zu2[:], glu[:])

            # ---- MM2: YT[h*128:(h+1)*128, :] = W2[:, hslice].T @ ACT ----
            ps2_pool = ctx.enter_context(tc.tile_pool(name="ps2", bufs=4, space="PSUM"))
            out_pool = ctx.enter_context(tc.tile_pool(name="outp", bufs=4))
            for h in range(NH):
                w2t = w2_tiles[h]
                for (s0, sz) in chunks:
                    p2 = ps2_pool.tile([128, sz], F32, tag="p2")
                    for i in range(NI):
                        nc.tensor.matmul(
                            p2[:],
                            w2t[:, i, :],
                            act_sb[:, i, s0:s0 + sz],
                            start=(i == 0),
                            stop=(i == NI - 1),
                        )
                    ot = out_pool.tile([128, sz], F32, tag="ot")
                    nc.vector.tensor_copy(ot[:], p2[:])
                    nc.sync.dma_start(out_d[h * 128:(h + 1) * 128, s0:s0 + sz], ot[:])

    nc.compile()
    return nc


def kernel(hidden_states, router_weight, router_bias, gate_up_proj,
           gate_up_bias, down_proj, down_bias):
    global last_exec_time_ns
    import os

    B, S, _ = hidden_states.shape
    T = B * S
    flat = np.ascontiguousarray(hidden_states.reshape(T, H), dtype=np.float32)

    # ---- Router (host): softmax + top-2, matching the reference math ----
    logits = flat @ router_weight.T.astype(np.float32) + router_bias
    m = logits.max(axis=-1, keepdims=True)
    ex = np.exp(logits - m)
    scores = ex / ex.sum(axis=-1, keepdims=True)
    topk_idx = np.argsort(-scores, axis=-1, kind="stable")[:, :TOP_K]
    topk_w = np.take_along_axis(scores, topk_idx, axis=-1)

    tok_lists, wgt_lists = [], []
    for e in range(E):
        sel = topk_idx == e
        toks = np.nonzero(sel.any(axis=1))[0]
        w_e = (topk_w * sel).sum(axis=1)[toks].astype(np.float32)
        tok_lists.append(toks)
        wgt_lists.append(w_e)

    Cmax = max(len(t) for t in tok_lists)
    C = max(256, -(-Cmax // 4) * 4)

    if C not in _prog_cache:
        _prog_cache[C] = _build_program(C)
    nc = _prog_cache[C]

    KH, NI, NJ, NH = H // 128, I // 128, I // 128, H // 128
    gup = np.asarray(gate_up_proj, dtype=np.float32)
    dwn = np.asarray(down_proj, dtype=np.float32)
    in_maps = []
    for e in range(E):
        toks = tok_lists[e]
        xt = np.zeros((H, C), ml_dtypes.bfloat16)
        xt[:, :len(toks)] = flat[toks].T.astype(ml_dtypes.bfloat16)
        # w1[m, p, k*128+c] = W1[k*128+p, m*128+c]
        w1 = np.ascontiguousarray(
            gup[e].reshape(KH, 128, 2 * NJ, 128).transpose(2, 1, 0, 3)
            .astype(ml_dtypes.bfloat16))
        # w2[h, p, i*128+c] = (W2/ALPHA)[i*128+p, h*128+c]
        w2 = np.ascontiguousarray(
            (dwn[e] * np.float32(1.0 / ALPHA))
            .reshape(NI, 128, NH, 128).transpose(2, 1, 0, 3)
            .astype(ml_dtypes.bfloat16))
        b1 = np.ascontiguousarray(
            np.asarray(gate_up_bias[e], dtype=np.float32).reshape(2 * NJ, 128).T)
        in_maps.append({"xt": xt, "w1": w1, "b1": b1, "w2": w2})

    trace = os.environ.get("KERNEL_TRACE") == "1"
    if trace:
        _install_ntff_hook()
    res = run_bass_kernel_spmd(nc, in_maps, core_ids=list(range(E)), trace=trace)
    last_exec_time_ns = res.exec_time_ns

    out = np.zeros((T, H), np.float32)
    for e in range(E):
        toks, w_e = tok_lists[e], wgt_lists[e]
        out[toks] += res.results[e]["out"][:, :len(toks)].T * w_e[:, None]
    # down_bias contribution: sum_k w_k * b2[e_k]
    if np.any(down_bias):
        out += (topk_w[:, :, None] * np.asarray(down_bias)[topk_idx]).sum(axis=1)
    return out.reshape(B, S, H).astype(np.float32)


# revision 13
# speedup vs baseline: 1.9731x; 1.9731x over previous
"""Expert-parallel MoE (top-2 of 8) kernel for 8 Trainium2 NeuronCores.

Strategy (per sharding hint): expert-parallel — expert e's FFN weights live on
core e. The (tiny) router runs on host; tokens are dispatched to their top-2
experts' cores as padded batches, each core runs its expert's gated-GLU FFN on
its batch (float32r matmuls, weights streamed from HBM), and the host applies
the routing weights and combines the per-expert partial sums.

Device layout is feature-major ([feature, token]) throughout so the contraction
dim is always on SBUF partitions and the gate_up bias is a per-partition scalar:

    XT[H=1024, C] --MM1--> GU[4096, C] --bias/clamp/silu--> ACT[2048, C]
       --MM2--> YT[1024, C]

The 1/1.702 from silu(1.702*z) = 1.702*z*sigmoid(1.702*z) is folded into
down_proj on the host. down_bias is combined on the host (it is outside the
matmuls: sum_k w_k * b2[e_k]). Weights are re-laid-out on the host so every
weight DMA moves multi-KB contiguous lines per partition.
"""

import ml_dtypes
import numpy as np

import concourse.bass as bass  # noqa: F401  (registers engines)
import concourse.mybir as mybir
import concourse.tile as tile
from concourse import bacc
from concourse.bass_utils import run_bass_kernel_spmd

ALPHA = 1.702
LIMIT = 7.0
TOP_K = 2
H = 1024
E = 8
I = 2048
F32 = mybir.dt.float32
F32R = mybir.dt.float32r
BF16 = mybir.dt.bfloat16

_prog_cache: dict = {}
last_exec_time_ns = None


def _install_ntff_hook():
    """Register the axon NTFF profiling hook if the image's antenv lacks it."""
    import sys, types  # noqa: PLC0415

    if "antenv.axon_hooks" in sys.modules:
        return
    try:
        import antenv  # noqa: PLC0415
        from trn_agent_boot.trn_boot import _ntff_profile_via_ctypes  # noqa: PLC0415

        hooks = types.ModuleType("antenv.axon_hooks")
        _h = [_ntff_profile_via_ctypes("/opt/axon/libaxon_pjrt.so")]
        hooks.set_axon_ntff_profile_hook = lambda h: _h.__setitem__(0, h)
        hooks.get_axon_ntff_profile_hook = lambda: _h[0]
        sys.modules["antenv.axon_hooks"] = hooks
        antenv.axon_hooks = hooks
    except Exception:
        pass


def _chunks(C):
    """Split C tokens into moving-operand chunks (<=512 each, >=256 when
    possible so float32r matmuls run at 1 cycle/row)."""
    n = max(1, -(-C // 512))
    base = C // n
    sizes = [base + (1 if i < C - base * n else 0) for i in range(n)]
    out, s = [], 0
    for sz in sizes:
        out.append((s, sz))
        s += sz
    return out


def _build_program(C):
    chunks = _chunks(C)
    add, mn, mx = mybir.AluOpType.add, mybir.AluOpType.min, mybir.AluOpType.max

    KH = H // 128   # 8 k-tiles over H (MM1 contraction)
    NI = I // 128   # 16 i-tiles over I (MM2 contraction)
    NJ = I // 128   # 16 gate col-tiles (up tile index = NJ + j)
    NH = H // 128   # 8 output h-tiles (MM2 stationary)

    nc = bacc.Bacc(
        "TRN2",
        target_bir_lowering=False,
        debug=False,
        enable_asserts=False,
        num_devices=E,
    )
    # host-prepared layouts (see kernel()):
    #   xt: X^T                        [H, C]
    #   w1: [m, p, k*128+c] m=0..31    (m<16: gate col-tile m; m>=16: up tile)
    #   w2: [h, p, i*128+c] h=0..7     (stationary tiles for MM2)
    #   b1: [p, m]                     per-partition bias for col-tile m
    xt_d = nc.dram_tensor("xt", [H, C], BF16, kind="ExternalInput").ap()
    w1_d = nc.dram_tensor("w1", [2 * NJ, 128, KH, 128], BF16, kind="ExternalInput").ap()
    b1_d = nc.dram_tensor("b1", [128, 2 * NJ], F32, kind="ExternalInput").ap()
    w2_d = nc.dram_tensor("w2", [NH, 128, NI, 128], BF16, kind="ExternalInput").ap()
    out_d = nc.dram_tensor("out", [H, C], F32, kind="ExternalOutput").ap()
    C2 = C // 2

    with tile.TileContext(nc) as tc:
        from contextlib import ExitStack

        with ExitStack() as ctx:
            const = ctx.enter_context(tc.tile_pool(name="const", bufs=1))
            xt_sb = const.tile([128, KH, C], BF16, tag="xt")
            for k in range(KH):
                # two DMAs per row-block: spread the fill over more DMA queues
                nc.sync.dma_start(
                    xt_sb[:, k, 0:C2], xt_d[k * 128:(k + 1) * 128, 0:C2])
                nc.sync.dma_start(
                    xt_sb[:, k, C2:C], xt_d[k * 128:(k + 1) * 128, C2:C])
            b1_sb = const.tile([128, 2 * NJ], F32, tag="b1")
            nc.sync.dma_start(b1_sb[:], b1_d[:])
            act_sb = const.tile([128, NI, C], BF16, tag="act")

            w1_pool = ctx.enter_context(tc.tile_pool(name="w1", bufs=4))
            w2_pool = ctx.enter_context(tc.tile_pool(name="w2", bufs=NH))
            ps_pool = ctx.enter_context(tc.tile_pool(name="ps", bufs=2, space="PSUM"))
            glu_pool = ctx.enter_context(tc.tile_pool(name="glu", bufs=3))
            w2_tiles = {}

            # ---- MM1 + GLU: act[:, j, :] for j in 0..15 ----
            for j in range(NJ):
                # weight loads are quartered (per-queue BW is ~25 GB/s) and
                # issued from the otherwise-idle GpSimd engine
                w1g = w1_pool.tile([128, KH, 128], BF16, tag="w1g")
                for q in range(2):
                    nc.gpsimd.dma_start(
                        w1g[:, 4 * q:4 * q + 4, :], w1_d[j, :, 4 * q:4 * q + 4, :])
                w1u = w1_pool.tile([128, KH, 128], BF16, tag="w1u")
                for q in range(2):
                    nc.sync.dma_start(
                        w1u[:, 4 * q:4 * q + 4, :], w1_d[NJ + j, :, 4 * q:4 * q + 4, :])
                if j % 2 == 1:
                    # stagger MM2 weight loads through the MM1 phase
                    h = j // 2
                    w2t = w2_pool.tile([128, NI, 128], BF16, tag="w2")
                    for q in range(4):
                        eng = nc.sync if q % 2 else nc.gpsimd
                        eng.dma_start(
                            w2t[:, 4 * q:4 * q + 4, :], w2_d[h, :, 4 * q:4 * q + 4, :])
                    w2_tiles[h] = w2t
                for (s0, sz) in chunks:
                    pg = ps_pool.tile([128, sz], F32, tag="pg")
                    for k in range(KH):
                        nc.tensor.matmul(
                            pg[:],
                            w1g[:, k, :],
                            xt_sb[:, k, s0:s0 + sz],
                            start=(k == 0),
                            stop=(k == KH - 1),
                        )
                    pu = ps_pool.tile([128, sz], F32, tag="pu")
                    for k in range(KH):
                        nc.tensor.matmul(
                            pu[:],
                            w1u[:, k, :],
                            xt_sb[:, k, s0:s0 + sz],
                            start=(k == 0),
                            stop=(k == KH - 1),
                        )
                    # gate: z = min(gu + b1g, LIMIT); glu' = silu(ALPHA*z) = ALPHA*glu
                    zg = glu_pool.tile([128, sz], F32, tag="zg")
                    nc.vector.tensor_scalar(
                        zg[:], pg[:], b1_sb[:, j:j + 1], LIMIT, op0=add, op1=mn
                    )
                    glu = glu_pool.tile([128, sz], F32, tag="glut")
                    nc.scalar.activation(
                        glu[:], zg[:], mybir.ActivationFunctionType.Silu, scale=ALPHA
                    )
                    # up: z = clip(gu + b1u, -LIMIT, LIMIT) + 1
                    zu = glu_pool.tile([128, sz], F32, tag="zu")
                    nc.vector.tensor_scalar(
                        zu[:], pu[:], b1_sb[:, NJ + j:NJ + j + 1], LIMIT,
                        op0=add, op1=mn,
                    )
                    zu2 = glu_pool.tile([128, sz], F32, tag="zu2")
                    nc.vector.tensor_scalar(
                        zu2[:], zu[:], -LIMIT, 1.0, op0=mx, op1=add
                    )
                    nc.vector.tensor_mul(act_sb[:, j, s0:s0 + sz], zu2[:], glu[:])

            # ---- MM2: YT[h*128:(h+1)*128, :] = W2[:, hslice].T @ ACT ----
            ps2_pool = ctx.enter_context(tc.tile_pool(name="ps2", bufs=4, space="PSUM"))
            out_pool = ctx.enter_context(tc.tile_pool(name="outp", bufs=4))
            for h in range(NH):
                w2t = w2_tiles[h]
                for (s0, sz) in chunks:
                    p2 = ps2_pool.tile([128, sz], F32, tag="p2")
                    for i in range(NI):
                        nc.tensor.matmul(
                            p2[:],
                            w2t[:, i, :],
                            act_sb[:, i, s0:s0 + sz],
                            start=(i == 0),
                            stop=(i == NI - 1),
                        )
                    ot = out_pool.tile([128, sz], F32, tag="ot")
                    nc.vector.tensor_copy(ot[:], p2[:])
                    nc.sync.dma_start(out_d[h * 128:(h + 1) * 128, s0:s0 + sz], ot[:])

    nc.compile()
    return nc


def kernel(hidden_states, router_weight, router_bias, gate_up_proj,
           gate_up_bias, down_proj, down_bias):
    global last_exec_time_ns
    import os

    B, S, _ = hidden_states.shape
    T = B * S
    flat = np.ascontiguousarray(hidden_states.reshape(T, H), dtype=np.float32)

    # ---- Router (host): softmax + top-2, matching the reference math ----
    logits = flat @ router_weight.T.astype(np.float32) + router_bias
    m = logits.max(axis=-1, keepdims=True)
    ex = np.exp(logits - m)
    scores = ex / ex.sum(axis=-1, keepdims=True)
    topk_idx = np.argsort(-scores, axis=-1, kind="stable")[:, :TOP_K]
    topk_w = np.take_along_axis(scores, topk_idx, axis=-1)

    tok_lists, wgt_lists = [], []
    for e in range(E):
        sel = topk_idx == e
        toks = np.nonzero(sel.any(axis=1))[0]
        w_e = (topk_w * sel).sum(axis=1)[toks].astype(np.float32)
        tok_lists.append(toks)
        wgt_lists.append(w_e)

    Cmax = max(len(t) for t in tok_lists)
    C = max(256, -(-Cmax // 4) * 4)

    if C not in _prog_cache:
        _prog_cache[C] = _build_program(C)
    nc = _prog_cache[C]

    KH, NI, NJ, NH = H // 128, I // 128, I // 128, H // 128
    gup = np.asarray(gate_up_proj, dtype=np.float32)
    dwn = np.asarray(down_proj, dtype=np.float32)
    in_maps = []
    for e in range(E):
        toks = tok_lists[e]
        xt = np.zeros((H, C), ml_dtypes.bfloat16)
        xt[:, :len(toks)] = flat[toks].T.astype(ml_dtypes.bfloat16)
        # w1[m, p, k*128+c] = W1[k*128+p, m*128+c]
        w1 = np.ascontiguousarray(
            gup[e].reshape(KH, 128, 2 * NJ, 128).transpose(2, 1, 0, 3)
            .astype(ml_dtypes.bfloat16))
        # w2[h, p, i*128+c] = (W2/ALPHA)[i*128+p, h*128+c]
        w2 = np.ascontiguousarray(
            (dwn[e] * np.float32(1.0 / ALPHA))
            .reshape(NI, 128, NH, 128).transpose(2, 1, 0, 3)
            .astype(ml_dtypes.bfloat16))
        b1 = np.ascontiguousarray(
            np.asarray(gate_up_bias[e], dtype=np.float32).reshape(2 * NJ, 128).T)
        in_maps.append({"xt": xt, "w1": w1, "b1": b1, "w2": w2})

    trace = os.environ.get("KERNEL_TRACE") == "1"
    if trace:
        _install_ntff_hook()
    res = run_bass_kernel_spmd(nc, in_maps, core_ids=list(range(E)), trace=trace)
    last_exec_time_ns = res.exec_time_ns

    out = np.zeros((T, H), np.float32)
    for e in range(E):
        toks, w_e = tok_lists[e], wgt_lists[e]
        out[toks] += res.results[e]["out"][:, :len(toks)].T * w_e[:, None]
    # down_bias contribution: sum_k w_k * b2[e_k]
    if np.any(down_bias):
        out += (topk_w[:, :, None] * np.asarray(down_bias)[topk_idx]).sum(axis=1)
    return out.reshape(B, S, H).astype(np.float32)


# revision 14
# speedup vs baseline: 2.0002x; 1.0137x over previous
"""Expert-parallel MoE (top-2 of 8) kernel for 8 Trainium2 NeuronCores.

Strategy (per sharding hint): expert-parallel — expert e's FFN weights live on
core e. The (tiny) router runs on host; tokens are dispatched to their top-2
experts' cores as padded batches, each core runs its expert's gated-GLU FFN on
its batch (float32r matmuls, weights streamed from HBM), and the host applies
the routing weights and combines the per-expert partial sums.

Device layout is feature-major ([feature, token]) throughout so the contraction
dim is always on SBUF partitions and the gate_up bias is a per-partition scalar:

    XT[H=1024, C] --MM1--> GU[4096, C] --bias/clamp/silu--> ACT[2048, C]
       --MM2--> YT[1024, C]

The 1/1.702 from silu(1.702*z) = 1.702*z*sigmoid(1.702*z) is folded into
down_proj on the host. down_bias is combined on the host (it is outside the
matmuls: sum_k w_k * b2[e_k]). Weights are re-laid-out on the host so every
weight DMA moves multi-KB contiguous lines per partition.
"""

import ml_dtypes
import numpy as np

import concourse.bass as bass  # noqa: F401  (registers engines)
import concourse.mybir as mybir
import concourse.tile as tile
from concourse import bacc
from concourse.bass_utils import run_bass_kernel_spmd

ALPHA = 1.702
LIMIT = 7.0
TOP_K = 2
H = 1024
E = 8
I = 2048
F32 = mybir.dt.float32
F32R = mybir.dt.float32r
BF16 = mybir.dt.bfloat16

_prog_cache: dict = {}
last_exec_time_ns = None


def _install_ntff_hook():
    """Register the axon NTFF profiling hook if the image's antenv lacks it."""
    import sys, types  # noqa: PLC0415

    if "antenv.axon_hooks" in sys.modules:
        return
    try:
        import antenv  # noqa: PLC0415
        from trn_agent_boot.trn_boot import _ntff_profile_via_ctypes  # noqa: PLC0415

        hooks = types.ModuleType("antenv.axon_hooks")
        _h = [_ntff_profile_via_ctypes("/opt/axon/libaxon_pjrt.so")]
        hooks.set_axon_ntff_profile_hook = lambda h: _h.__setitem__(0, h)
        hooks.get_axon_ntff_profile_hook = lambda: _h[0]
        sys.modules["antenv.axon_hooks"] = hooks
        antenv.axon_hooks = hooks
    except Exception:
        pass


def _chunks(C):
    """Split C tokens into moving-operand chunks (<=512 each, >=256 when
    possible so float32r matmuls run at 1 cycle/row)."""
    n = max(1, -(-C // 512))
    base = C // n
    sizes = [base + (1 if i < C - base * n else 0) for i in range(n)]
    out, s = [], 0
    for sz in sizes:
        out.append((s, sz))
        s += sz
    return out


def _build_program(C):
    chunks = _chunks(C)
    add, mn, mx = mybir.AluOpType.add, mybir.AluOpType.min, mybir.AluOpType.max

    KH = H // 128   # 8 k-tiles over H (MM1 contraction)
    NI = I // 128   # 16 i-tiles over I (MM2 contraction)
    NJ = I // 128   # 16 gate col-tiles (up tile index = NJ + j)
    NH = H // 128   # 8 output h-tiles (MM2 stationary)

    nc = bacc.Bacc(
        "TRN2",
        target_bir_lowering=False,
        debug=False,
        enable_asserts=False,
        num_devices=E,
    )
    # host-prepared layouts (see kernel()):
    #   xt: X^T                        [H, C]
    #   w1: [m, p, k*128+c] m=0..31    (m<16: gate col-tile m; m>=16: up tile)
    #   w2: [h, p, i*128+c] h=0..7     (stationary tiles for MM2)
    #   b1: [p, m]                     per-partition bias for col-tile m
    xt_d = nc.dram_tensor("xt", [H, C], BF16, kind="ExternalInput").ap()
    w1_d = nc.dram_tensor("w1", [2 * NJ, 128, KH, 128], BF16, kind="ExternalInput").ap()
    b1_d = nc.dram_tensor("b1", [128, 2 * NJ], F32, kind="ExternalInput").ap()
    w2_d = nc.dram_tensor("w2", [NH, 128, NI, 128], BF16, kind="ExternalInput").ap()
    out_d = nc.dram_tensor("out", [H, C], F32, kind="ExternalOutput").ap()
    C2 = C // 2

    with tile.TileContext(nc) as tc:
        from contextlib import ExitStack

        with ExitStack() as ctx:
            const = ctx.enter_context(tc.tile_pool(name="const", bufs=1))
            xt_sb = const.tile([128, KH, C], BF16, tag="xt")
            for k in range(KH):
                # two DMAs per row-block: spread the fill over more DMA queues
                nc.sync.dma_start(
                    xt_sb[:, k, 0:C2], xt_d[k * 128:(k + 1) * 128, 0:C2])
                nc.sync.dma_start(
                    xt_sb[:, k, C2:C], xt_d[k * 128:(k + 1) * 128, C2:C])
            b1_sb = const.tile([128, 2 * NJ], F32, tag="b1")
            nc.sync.dma_start(b1_sb[:], b1_d[:])
            act_sb = const.tile([128, NI, C], BF16, tag="act")

            w1_pool = ctx.enter_context(tc.tile_pool(name="w1", bufs=4))
            w2_pool = ctx.enter_context(tc.tile_pool(name="w2", bufs=NH))
            ps_pool = ctx.enter_context(tc.tile_pool(name="ps", bufs=2, space="PSUM"))
            glu_pool = ctx.enter_context(tc.tile_pool(name="glu", bufs=3))
            w2_tiles = {}

            # ---- MM1 + GLU: act[:, j, :] for j in 0..15 ----
            for j in range(NJ):
                # weight loads are quartered (per-queue BW is ~25 GB/s) and
                # issued from the otherwise-idle GpSimd engine
                w1g = w1_pool.tile([128, KH, 128], BF16, tag="w1g")
                for q in range(2):
                    nc.gpsimd.dma_start(
                        w1g[:, 4 * q:4 * q + 4, :], w1_d[j, :, 4 * q:4 * q + 4, :])
                w1u = w1_pool.tile([128, KH, 128], BF16, tag="w1u")
                for q in range(2):
                    nc.gpsimd.dma_start(
                        w1u[:, 4 * q:4 * q + 4, :], w1_d[NJ + j, :, 4 * q:4 * q + 4, :])
                if j % 2 == 1:
                    # stagger MM2 weight loads through the MM1 phase
                    h = j // 2
                    w2t = w2_pool.tile([128, NI, 128], BF16, tag="w2")
                    for q in range(4):
                        nc.gpsimd.dma_start(
                            w2t[:, 4 * q:4 * q + 4, :], w2_d[h, :, 4 * q:4 * q + 4, :])
                    w2_tiles[h] = w2t
                for (s0, sz) in chunks:
                    pg = ps_pool.tile([128, sz], F32, tag="pg")
                    for k in range(KH):
                        nc.tensor.matmul(
                            pg[:],
                            w1g[:, k, :],
                            xt_sb[:, k, s0:s0 + sz],
                            start=(k == 0),
                            stop=(k == KH - 1),
                        )
                    pu = ps_pool.tile([128, sz], F32, tag="pu")
                    for k in range(KH):
                        nc.tensor.matmul(
                            pu[:],
                            w1u[:, k, :],
                            xt_sb[:, k, s0:s0 + sz],
                            start=(k == 0),
                            stop=(k == KH - 1),
                        )
                    # gate: z = min(gu + b1g, LIMIT); glu' = silu(ALPHA*z) = ALPHA*glu
                    zg = glu_pool.tile([128, sz], F32, tag="zg")
                    nc.vector.tensor_scalar(
                        zg[:], pg[:], b1_sb[:, j:j + 1], LIMIT, op0=add, op1=mn
                    )
                    glu = glu_pool.tile([128, sz], F32, tag="glut")
                    nc.scalar.activation(
                        glu[:], zg[:], mybir.ActivationFunctionType.Silu, scale=ALPHA
                    )
                    # up: z = clip(gu + b1u, -LIMIT, LIMIT) + 1
                    zu = glu_pool.tile([128, sz], F32, tag="zu")
                    nc.vector.tensor_scalar(
                        zu[:], pu[:], b1_sb[:, NJ + j:NJ + j + 1], LIMIT,
                        op0=add, op1=mn,
                    )
                    zu2 = glu_pool.tile([128, sz], F32, tag="zu2")
                    nc.vector.tensor_scalar(
                        zu2[:], zu[:], -LIMIT, 1.0, op0=mx, op1=add
                    )
                    nc.vector.tensor_mul(act_sb[:, j, s0:s0 + sz], zu2[:], glu[:])

            # ---- MM2: YT[h*128:(h+1)*128, :] = W2[:, hslice].T @ ACT ----
            ps2_pool = ctx.enter_context(tc.tile_pool(name="ps2", bufs=4, space="PSUM"))
            out_pool = ctx.enter_context(tc.tile_pool(name="outp", bufs=4))
            for h in range(NH):
                w2t = w2_tiles[h]
                for (s0, sz) in chunks:
                    p2 = ps2_pool.tile([128, sz], F32, tag="p2")
                    for i in range(NI):
                        nc.tensor.matmul(
                            p2[:],
                            w2t[:, i, :],
                            act_sb[:, i, s0:s0 + sz],
                            start=(i == 0),
                            stop=(i == NI - 1),
                        )
                    ot = out_pool.tile([128, sz], F32, tag="ot")
                    nc.vector.tensor_copy(ot[:], p2[:])
                    nc.sync.dma_start(out_d[h * 128:(h + 1) * 128, s0:s0 + sz], ot[:])

    nc.compile()
    return nc


def kernel(hidden_states, router_weight, router_bias, gate_up_proj,
           gate_up_bias, down_proj, down_bias):
    global last_exec_time_ns
    import os

    B, S, _ = hidden_states.shape
    T = B * S
    flat = np.ascontiguousarray(hidden_states.reshape(T, H), dtype=np.float32)

    # ---- Router (host): softmax + top-2, matching the reference math ----
    logits = flat @ router_weight.T.astype(np.float32) + router_bias
    m = logits.max(axis=-1, keepdims=True)
    ex = np.exp(logits - m)
    scores = ex / ex.sum(axis=-1, keepdims=True)
    topk_idx = np.argsort(-scores, axis=-1, kind="stable")[:, :TOP_K]
    topk_w = np.take_along_axis(scores, topk_idx, axis=-1)

    tok_lists, wgt_lists = [], []
    for e in range(E):
        sel = topk_idx == e
        toks = np.nonzero(sel.any(axis=1))[0]
        w_e = (topk_w * sel).sum(axis=1)[toks].astype(np.float32)
        tok_lists.append(toks)
        wgt_lists.append(w_e)

    Cmax = max(len(t) for t in tok_lists)
    C = max(256, -(-Cmax // 4) * 4)

    if C not in _prog_cache:
        _prog_cache[C] = _build_program(C)
    nc = _prog_cache[C]

    KH, NI, NJ, NH = H // 128, I // 128, I // 128, H // 128
    gup = np.asarray(gate_up_proj, dtype=np.float32)
    dwn = np.asarray(down_proj, dtype=np.float32)
    in_maps = []
    for e in range(E):
        toks = tok_lists[e]
        xt = np.zeros((H, C), ml_dtypes.bfloat16)
        xt[:, :len(toks)] = flat[toks].T.astype(ml_dtypes.bfloat16)
        # w1[m, p, k*128+c] = W1[k*128+p, m*128+c]
        w1 = np.ascontiguousarray(
            gup[e].reshape(KH, 128, 2 * NJ, 128).transpose(2, 1, 0, 3)
            .astype(ml_dtypes.bfloat16))
        # w2[h, p, i*128+c] = (W2/ALPHA)[i*128+p, h*128+c]
        w2 = np.ascontiguousarray(
            (dwn[e] * np.float32(1.0 / ALPHA))
            .reshape(NI, 128, NH, 128).transpose(2, 1, 0, 3)
            .astype(ml_dtypes.bfloat16))
        b1 = np.ascontiguousarray(
            np.asarray(gate_up_bias[e], dtype=np.float32).reshape(2 * NJ, 128).T)
        in_maps.append({"xt": xt, "w1": w1, "b1": b1, "w2": w2})

    trace = os.environ.get("KERNEL_TRACE") == "1"
    if trace:
        _install_ntff_hook()
    res = run_bass_kernel_spmd(nc, in_maps, core_ids=list(range(E)), trace=trace)
    last_exec_time_ns = res.exec_time_ns

    out = np.zeros((T, H), np.float32)
    for e in range(E):
        toks, w_e = tok_lists[e], wgt_lists[e]
        out[toks] += res.results[e]["out"][:, :len(toks)].T * w_e[:, None]
    # down_bias contribution: sum_k w_k * b2[e_k]
    if np.any(down_bias):
        out += (topk_w[:, :, None] * np.asarray(down_bias)[topk_idx]).sum(axis=1)
    return out.reshape(B, S, H).astype(np.float32)
